# revision 29
# baseline (speedup 1.0000x reference)
"""Trainium2 Bass kernel for nn_Attention_11192684774105.

Reference computation (B=4, C=128, H=W=64, N=4096, 2 heads, key_dim=32,
head_dim=64):
    qkv  = conv1x1(x, w_qkv)                    # [B,256,H,W]
    q,k,v split per head; attn = softmax(q^T k / sqrt(32)) over keys
    out  = v @ attn^T  (+ depthwise3x3(v, w_pe)) -> conv1x1(w_proj)

Sharding: 8 cores = (batch b, row-half) pairs.  Each core computes both
heads for 2048 query positions (32 image rows) of one sample; keys/values
span the full 4096 positions.  Output is a pure concatenation.

Per-core algorithm (bf16 compute, fp32 PSUM accumulation):
  - Inputs are pre-quantized to bf16 on the host (halves DMA traffic).
  - K/V come from a row-rotated copy of x so the depthwise-conv halo rows
    sit at fixed positions (attention is invariant to a consistent
    permutation of the key axis).
  - S^T = K^T Q is computed 512 queries per matmul with the 32-deep
    contraction placed on a rotating PE row-quadrant (tile_position);
    the rotation lets every weight load overlap the previous matmul
    (pinning one quadrant measures 2.2x slower).
  - exp() runs 1024 elements/op out of PSUM; most blocks on the scalar
    engine (|logits| < 3, no max subtraction), ~25% on the vector engine
    via a Schraudolph bit-trick straight into bf16 bits (the softmax
    ratio cancels most of the approximation error).  The denominator
    reciprocal row is broadcast across partitions with a ones-column
    matmul on the 64-row PE quadrant.
  - The softmax denominator comes from an extra ones-column in the V^T
    stationary operand of the second matmul (row 64 of the accumulator).
  - V^T blocks are produced directly as matmuls (x_blk^T @ Wv^T), not via
    PE transposes.
  - The depthwise conv + output projection are fused into 10 accumulating
    1x1 matmuls per query chunk at the stream tail.
  - Input DMA loads are software-pipelined: a preamble load before the
    timing loop, in-body reloads placed right after the last consumer.
    The qkv chunks gating the next iteration's first S^T matmuls are
    produced at the tail of segment 3, so under tc.For_i each iteration's
    attention stream starts immediately.
"""

import os

import numpy as np
import ml_dtypes

import concourse.bass as bass
import concourse.mybir as mybir
import concourse.tile as tile
from concourse import bacc
from concourse.bass_utils import run_bass_kernel_spmd

F32 = mybir.dt.float32
F32R = mybir.dt.float32r
BF16 = mybir.dt.bfloat16
U16 = mybir.dt.uint16
AF = mybir.ActivationFunctionType
ALU = mybir.AluOpType

B, C, H, W = 4, 128, 64, 64
N = H * W                    # 4096
NHALF = N // 2               # 2048 query positions per core
SCALE = 32 ** (-0.5)
MB = N // 128                # 32 key blocks of 128

# Schraudolph exp -> bf16 bits: exp(SCALE*s) ~ bitcast_u16(round(s*A + B))
A_SCH = SCALE * 128.0 / float(np.log(2.0))
B_SCH = 127.0 * 128.0 - 8.0

# wpack column-block indices (each block is 128 bf16 columns)
WQ0, WQ1, WK0, WK1, WV01, WPROJT, MS0 = 0, 1, 2, 3, 4, 5, 6
NWBLK = 15


def _alloc(tc):
    nc = tc.nc
    const = tc.alloc_tile_pool(name="const", bufs=1)
    epool = tc.alloc_tile_pool(name="epool", bufs=5)
    npool = tc.alloc_tile_pool(name="npool", bufs=2)
    psum = tc.alloc_tile_pool(name="psum", bufs=1, space="PSUM")
    pools = dict(const=const, epool=epool, npool=npool, psum=psum)

    t = {}
    specs = [
        ("wp", [128, NWBLK * 128], BF16), ("cst", [128, 4], F32),
        ("xr", [128, N], BF16),
        ("krep0", [128, N], BF16), ("krep1", [128, N], BF16),
        ("qrep0", [128, NHALF], BF16), ("qrep1", [128, NHALF], BF16),
        ("vchan", [128, N], BF16), ("vaugT", [128, MB * 130], BF16),
        ("vpad", [128, 34 * 66], BF16), ("attn", [128, NHALF], BF16),
        ("outsb", [128, NHALF], F32), ("ones64", [128, 64], F32R),
    ]
    for name, shape, dt in specs:
        t[name] = const.tile(shape, dt, name=name)
    return pools, t


def _emit_preamble(tc, t, io, pools):
    nc = tc.nc
    no_dma = os.environ.get("KPROBE", "") == "no_dma"
    if not no_dma:
        nc.sync.dma_start(out=t["wp"][:, 0:5 * 128], in_=io["wpack"][:, 0:5 * 128])
        nc.sync.dma_start(out=t["wp"][:, 5 * 128:NWBLK * 128],
                          in_=io["wpack"][:, 5 * 128:NWBLK * 128])
        nc.sync.dma_start(out=t["cst"], in_=io["cst"])
        for j in range(4):
            nc.sync.dma_start(out=t["xr"][:, j * 1024:(j + 1) * 1024],
                              in_=io["x_rot"][:, j * 1024:(j + 1) * 1024])

    one_col = t["cst"][:, 2:3]
    zero_col = t["cst"][:, 3:4]
    # Pin the exp table set before any ACT op picks a different one.
    actwarm = pools["npool"].tile([128, 1], F32, tag="actwarm", name="actwarm")
    nc.scalar.activation(out=actwarm, in_=one_col, func=AF.Exp, scale=1.0)
    # ones columns of the augmented-V^T stationary (cols 64,129 mod 130)
    vaug4 = t["vaugT"].rearrange("p (mb c) -> p mb c", mb=MB, c=130)
    nc.vector.tensor_copy(out=vaug4[:, :, 64:130:65],
                          in_=one_col.broadcast_to([128, MB, 2]))
    # zero left/right borders of the padded-V image (cols 0,65 mod 66)
    vpadr = t["vpad"].rearrange("p (r c) -> p r c", r=34, c=66)
    nc.vector.tensor_copy(out=vpadr[:, :, 0:66:65],
                          in_=zero_col.broadcast_to([128, 34, 2]))
    nc.vector.tensor_copy(out=t["ones64"], in_=one_col.broadcast_to([128, 64]))
    # first-iteration qkv chunks that gate the first S^T matmuls
    for blk, dst, j, off in [(WK0, t["krep0"], 0, 0), (WQ0, t["qrep0"], 0, 64),
                             (WQ0, t["qrep0"], 1, 64)]:
        pa = pools["psum"].tile([128, 512], F32, tag="s", bufs=3, name="pa")
        nc.tensor.matmul(pa, lhsT=t["wp"][:, blk * 128:(blk + 1) * 128],
                         rhs=t["xr"][:, off + j * 512:off + (j + 1) * 512],
                         start=True, stop=True)
        nc.vector.tensor_copy(out=dst[:, j * 512:(j + 1) * 512], in_=pa)


def _emit_body(tc, t, io, pools):
    nc = tc.nc
    psum, epool, npool = pools["psum"], pools["epool"], pools["npool"]
    probe = os.environ.get("KPROBE", "")
    kdve8 = int(os.environ.get("KDVE8", "2"))   # of 8 pair-tiles -> DVE exp
    no_dma = probe == "no_dma"
    skip_attn = probe == "no_attn"
    skip_oacc = probe in ("no_oacc", "no_exp")
    skip_exp = probe == "no_exp"

    wp, cst = t["wp"], t["cst"]
    xr, vchan, vaugT, vpad = t["xr"], t["vchan"], t["vaugT"], t["vpad"]
    attn, outsb, ones64 = t["attn"], t["outsb"], t["ones64"]

    def wblk(i):
        return wp[:, i * 128:(i + 1) * 128]

    # ---- phase A prologue: K0/Q0 so the attention stream can start -------
    def emit_pa(blk, src, dst, j, off=0):
        pa = psum.tile([128, 512], F32, tag="s", bufs=3, name="pa")
        nc.tensor.matmul(pa, lhsT=wblk(blk),
                         rhs=src[:, off + j * 512:off + (j + 1) * 512],
                         start=True, stop=True)
        nc.vector.tensor_copy(out=dst[:, j * 512:(j + 1) * 512], in_=pa)

    # The qkv conv is interleaved into the attention stream's engine slack.
    # The three chunks gating the first S^T matmuls (krep0 c0, qrep0 c0/c1)
    # are produced for the NEXT loop iteration at the tail of segment 3 (the
    # preamble provides them for the first iteration), so each iteration's
    # attention stream starts immediately.
    inj = {0: {}, 1: {}, 2: {}, 3: {}}
    for mb in range(7):
        inj[0][mb] = (WK0, xr, t["krep0"], mb + 1, 0)
    inj[0][24] = (WQ0, xr, t["qrep0"], 2, 64)
    inj[0][26] = (WQ0, xr, t["qrep0"], 3, 64)
    inj[1][0] = (WK1, xr, t["krep1"], 0, 0)
    inj[1][2] = (WQ1, xr, t["qrep1"], 0, 64)
    inj[1][4] = (WQ1, xr, t["qrep1"], 1, 64)
    for j in range(7):
        inj[2][2 * j + 1] = (WK1, xr, t["krep1"], j + 1, 0)
    inj[2][15] = (WQ1, xr, t["qrep1"], 2, 64)
    inj[2][17] = (WQ1, xr, t["qrep1"], 3, 64)
    for j in range(4):
        inj[2][19 + 2 * j] = (WV01, xr, vchan, j, 0)
    for j in range(4):
        inj[3][2 * j] = (WV01, xr, vchan, 4 + j, 0)
    inj[3][10] = (WK0, xr, t["krep0"], 0, 0)
    inj[3][12] = (WQ0, xr, t["qrep0"], 0, 64)
    inj[3][14] = (WQ0, xr, t["qrep0"], 1, 64)

    def emit_pt(mb):
        # V^T block straight from x: pt = x_blk^T @ Wv^T  -> [keys, dims]
        pt = psum.tile([128, 128], F32, tag="s", bufs=3, name="pt")
        nc.tensor.matmul(pt, lhsT=xr[:, mb * 128:(mb + 1) * 128],
                         rhs=wblk(WV01), start=True, stop=True)
        dst = vaugT[:, mb * 130:mb * 130 + 130].rearrange(
            "p (a b) -> p a b", a=2, b=65)[:, :, 0:64]
        src = pt.rearrange("p (a b) -> p a b", a=2, b=64)
        nc.vector.tensor_copy(out=dst, in_=src)

    vpad3 = vpad.rearrange("p (r c) -> p r c", r=34, c=66)

    if skip_attn or skip_oacc:
        # keep attn written for the probe builds (timing only)
        nc.vector.tensor_copy(
            out=attn, in_=cst[:, 3:4].broadcast_to([128, NHALF]))

    # ---- phase C: attention, one continuous S^T -> exp -> O stream -------
    # Four (head, query-half) segments share a single software pipeline so
    # the scalar engine never drains between segments.  The 32-deep S^T
    # contraction rotates the PE row-quadrant every matmul (rg=(2mb+cl)%4)
    # so consecutive S^T matmuls run concurrently on different row bands.
    # The kdve schedule sends part of each segment's exp() to the vector
    # engine (Schraudolph bit-trick); segment 0 keeps the vector engine
    # free for the qkv/V^T PSUM evacuations injected into its slack.
    LAG = int(os.environ.get("KLAG", "3"))
    kdve_sched = [0, kdve8, (3 * kdve8 + 1) // 2, (3 * kdve8 + 1) // 2]
    if os.environ.get("KDVE_SCHED"):
        kdve_sched = [int(v) for v in os.environ["KDVE_SCHED"].split(",")]
    segs = [] if skip_attn else [(h, p) for h in range(2) for p in range(2)]
    oaccs = {}
    pend = []

    def normalize(si):
        # rows 0:64 / row 64 (the ones-column accumulation)
        h, npass = segs[si]
        c0 = 2 * npass
        for cl in range(2):
            rec = npool.tile([128, 512], F32, tag="rec", name="rec")
            nc.vector.reciprocal(out=rec[64:65, :],
                                 in_=oaccs[si][cl][64:65, :])
            rb = npool.tile([128, 512], F32, tag="rb", name="rb")
            nc.gpsimd.partition_broadcast(out_ap=rb[0:64, :],
                                          in_=rec[64:65, :])
            nc.vector.tensor_mul(
                out=attn[h * 64:(h + 1) * 64,
                         (c0 + cl) * 512:(c0 + cl + 1) * 512],
                in0=oaccs[si][cl][0:64, :], in1=rb[0:64, :])

    def flush_o(si, mb, et):
        h = segs[si][0]
        for cl in range(2):
            nc.tensor.matmul(
                oaccs[si][cl][0:65, :],
                lhsT=vaugT[:, mb * 130 + h * 65:mb * 130 + h * 65 + 65],
                rhs=et[:, cl * 512:(cl + 1) * 512],
                start=(mb == 0), stop=(mb == MB - 1))
        if mb == MB - 1:
            normalize(si)

    for si, (h, npass) in enumerate(segs):
        krep = (t["krep0"], t["krep1"])[h]
        qrep = (t["qrep0"], t["qrep1"])[h]
        c0 = 2 * npass
        kdve = kdve_sched[si] if not skip_oacc else kdve8
        if not skip_oacc:
            oaccs[si] = [psum.tile([128, 512], F32, tag="o", bufs=2,
                                   name=f"oacc{cl}") for cl in range(2)]
        for mb in range(MB):
            # injected qkv / V^T work (phase A spread over the stream)
            if si == 0:
                emit_pt(mb)
            if mb in inj[si]:
                emit_pa(*inj[si][mb])
            if si == 3 and mb == 8:
                # vchan complete: padded-V image for the depthwise conv
                nc.vector.tensor_copy(out=vpad3[:, 1:33, 1:65],
                                      in_=vchan[:, 64:64 + 32 * 64])
                nc.vector.tensor_scalar(
                    out=vpad3[:, 0, 1:65], in0=vchan[:, 0:64],
                    scalar1=cst[:, 0:1], scalar2=None, op0=ALU.mult)
                nc.vector.tensor_scalar(
                    out=vpad3[:, 33, 1:65], in0=vchan[:, 33 * 64:34 * 64],
                    scalar1=cst[:, 1:2], scalar2=None, op0=ALU.mult)
            st = psum.tile([128, 1024], F32, tag="s", bufs=3, name="st")
            for cl in range(2):
                rg = 0 if os.environ.get("KRG0") else (2 * mb + cl) % 4
                nc.tensor.matmul(
                    st[:, cl * 512:(cl + 1) * 512],
                    lhsT=krep[32 * rg:32 * (rg + 1),
                              mb * 128:(mb + 1) * 128],
                    rhs=qrep[32 * rg:32 * (rg + 1),
                             (c0 + cl) * 512:(c0 + cl + 1) * 512],
                    start=True, stop=True,
                    tile_position=(32 * rg, 0))
            if skip_exp:
                continue
            et = epool.tile([128, 1024], BF16, tag="e", name="et")
            if ((mb + 1) * kdve) // 8 > (mb * kdve) // 8:
                with nc.allow_low_precision(reason="schraudolph exp"):
                    nc.vector.tensor_scalar(
                        out=et.bitcast(U16), in0=st,
                        scalar1=A_SCH, scalar2=B_SCH,
                        op0=ALU.mult, op1=ALU.add)
            else:
                nc.scalar.activation(out=et, in_=st, func=AF.Exp,
                                     scale=SCALE)
            if skip_oacc:
                continue
            pend.append((si, mb, et))
            if len(pend) > LAG:
                flush_o(*pend.pop(0))
        if si == 1 and not no_dma:
            # x / qkv weights reload; later readers get identical values
            nc.sync.dma_start(out=xr, in_=io["x_rot"])
            nc.sync.dma_start(out=wp[:, 0:5 * 128],
                              in_=io["wpack"][:, 0:5 * 128])
    for pe_ in pend:
        flush_o(*pe_)

    # ---- phase E: fused depthwise-conv + projection ----------------------
    shifts = [(dy, dx) for dy in (-1, 0, 1) for dx in (-1, 0, 1)]
    for cpair in range(2):
        psf = {}
        for cch in (2 * cpair, 2 * cpair + 1):
            psf[cch] = psum.tile([128, 512], F32, tag="s", bufs=3,
                                 name=f"psf{cch}")
        for widx in range(10):
            for cch in (2 * cpair, 2 * cpair + 1):
                if widx == 0:
                    lhsT = wblk(WPROJT)
                    rhs = attn[:, cch * 512:(cch + 1) * 512]
                else:
                    dy, dx = shifts[widx - 1]
                    lhsT = wblk(MS0 + widx - 1)
                    r0 = 1 + dy + 8 * cch
                    rhs = vpad3[:, r0:r0 + 8, 1 + dx:65 + dx]
                nc.tensor.matmul(psf[cch], lhsT=lhsT, rhs=rhs,
                                 start=(widx == 0), stop=(widx == 9))
        for cch in (2 * cpair, 2 * cpair + 1):
            sl = slice(cch * 512, (cch + 1) * 512)
            nc.vector.tensor_copy(out=outsb[:, sl], in_=psf[cch])
            if not no_dma:
                nc.sync.dma_start(out=io["out"][:, sl], in_=outsb[:, sl])

    # ---- reload proj/shift weights + consts for the next iteration -------
    if not no_dma:
        nc.sync.dma_start(out=wp[:, 5 * 128:NWBLK * 128],
                          in_=io["wpack"][:, 5 * 128:NWBLK * 128])
        nc.sync.dma_start(out=cst, in_=io["cst"])


def build_nc(reps=1):
    nc = bacc.Bacc(trn_type="TRN2", target_bir_lowering=False)
    io = {
        "wpack": nc.dram_tensor("wpack", [128, NWBLK * 128], BF16,
                                kind="ExternalInput").ap(),
        "cst": nc.dram_tensor("cst", [128, 4], F32,
                              kind="ExternalInput").ap(),
        "x_rot": nc.dram_tensor("x_rot", [128, N], BF16,
                                kind="ExternalInput").ap(),
        "out": nc.dram_tensor("out", [128, NHALF], F32,
                              kind="ExternalOutput").ap(),
    }
    with tile.TileContext(nc) as tc:
        pools, t = _alloc(tc)
        _emit_preamble(tc, t, io, pools)
        if reps == 1:
            _emit_body(tc, t, io, pools)
        else:
            with tc.For_i(0, reps, 1):
                _emit_body(tc, t, io, pools)
        for p in reversed(list(pools.values())):
            p.release()
    nc.compile()
    return nc


def host_prep(x, w_qkv, w_pe, w_proj):
    """Build the 8 per-core input maps from the full problem inputs."""
    bf16 = ml_dtypes.bfloat16
    x = np.ascontiguousarray(x, dtype=np.float32)
    wq = np.asarray(w_qkv, dtype=np.float32)[:, :, 0, 0]      # [256,128]
    wpe = np.asarray(w_pe, dtype=np.float32)[:, 0]            # [128,3,3]
    wpj = np.asarray(w_proj, dtype=np.float32)[:, :, 0, 0]    # [128,128]

    blocks = []
    for h in range(2):
        blocks.append(np.tile(wq[h * 128:h * 128 + 32], (4, 1)).T)       # WQh
    for h in range(2):
        blocks.append(np.tile(wq[h * 128 + 32:h * 128 + 64], (4, 1)).T)  # WKh
    blocks.insert(4, np.concatenate(
        [wq[64:128], wq[192:256]], axis=0).T)                 # WV01
    blocks.append(wpj.T)                                      # WPROJT
    for dy in (-1, 0, 1):
        for dx in (-1, 0, 1):
            blocks.append((wpj * wpe[:, dy + 1, dx + 1][None, :]).T)
    wpack = np.concatenate(blocks, axis=1).astype(bf16)       # [128, 15*128]

    in_maps = []
    for core in range(8):
        b, half = core // 2, core % 2
        y0 = 32 * half
        cst = np.zeros((128, 4), np.float32)
        cst[:, 0] = 1.0 if half == 1 else 0.0     # top halo valid?
        cst[:, 1] = 1.0 if half == 0 else 0.0     # bottom halo valid?
        cst[:, 2] = 1.0
        x_rot = np.roll(x[b], 1 - y0, axis=1).reshape(128, N).astype(bf16)
        in_maps.append({
            "wpack": np.ascontiguousarray(wpack),
            "cst": cst,
            "x_rot": np.ascontiguousarray(x_rot),
        })
    return in_maps


def assemble(results):
    out = np.zeros((B, C, H, W), np.float32)
    for core in range(8):
        b, half = core // 2, core % 2
        out[b, :, 32 * half:32 * half + 32, :] = \
            results[core]["out"].reshape(C, 32, W)
    return out


_NC_CACHE = {}


def _get_nc(reps=1):
    if reps not in _NC_CACHE:
        _NC_CACHE[reps] = build_nc(reps)
    return _NC_CACHE[reps]


def run(x, w_qkv, w_pe, w_proj, reps=1, **spmd_kwargs):
    nc = _get_nc(reps)
    in_maps = host_prep(x, w_qkv, w_pe, w_proj)
    res = run_bass_kernel_spmd(nc, in_maps, core_ids=list(range(8)),
                               **spmd_kwargs)
    return assemble(res.results), res


def kernel(x, w_qkv, w_pe, w_proj):
    out, _ = run(x, w_qkv, w_pe, w_proj)
    return out


# revision 30
# speedup vs baseline: 1.1225x; 1.1225x over previous
"""Trainium2 Bass kernel for nn_Attention_11192684774105.

Reference computation (B=4, C=128, H=W=64, N=4096, 2 heads, key_dim=32,
head_dim=64):
    qkv  = conv1x1(x, w_qkv)                    # [B,256,H,W]
    q,k,v split per head; attn = softmax(q^T k / sqrt(32)) over keys
    out  = v @ attn^T  (+ depthwise3x3(v, w_pe)) -> conv1x1(w_proj)

Sharding: 8 cores = (batch b, row-half) pairs.  Each core computes both
heads for 2048 query positions (32 image rows) of one sample; keys/values
span the full 4096 positions.  Output is a pure concatenation.

Per-core algorithm (bf16 compute, fp32 PSUM accumulation):
  - Inputs are pre-quantized to bf16 on the host (halves DMA traffic).
  - K/V come from a row-rotated copy of x so the depthwise-conv halo rows
    sit at fixed positions (attention is invariant to a consistent
    permutation of the key axis).
  - S^T = K^T Q is computed 512 queries per matmul with the 32-deep
    contraction placed on a rotating PE row-quadrant (tile_position);
    the rotation lets every weight load overlap the previous matmul
    (pinning one quadrant measures 2.2x slower).
  - exp() runs 1024 elements/op out of PSUM; most blocks on the scalar
    engine (|logits| < 3, no max subtraction), ~25% on the vector engine
    via a Schraudolph bit-trick straight into bf16 bits (the softmax
    ratio cancels most of the approximation error).  The denominator
    reciprocal row is broadcast across partitions with a ones-column
    matmul on the 64-row PE quadrant.
  - The softmax denominator comes from an extra ones-column in the V^T
    stationary operand of the second matmul (row 64 of the accumulator).
  - V^T blocks are produced directly as matmuls (x_blk^T @ Wv^T), not via
    PE transposes.
  - The depthwise conv + output projection are fused into 10 accumulating
    1x1 matmuls per query chunk at the stream tail.
  - Input DMA loads are software-pipelined: a preamble load before the
    timing loop, in-body reloads placed right after the last consumer.
    The qkv chunks gating the next iteration's first S^T matmuls are
    produced at the tail of segment 3, so under tc.For_i each iteration's
    attention stream starts immediately.
"""

import os

import numpy as np
import ml_dtypes

import concourse.bass as bass
import concourse.mybir as mybir
import concourse.tile as tile
from concourse import bacc
from concourse.bass_utils import run_bass_kernel_spmd

F32 = mybir.dt.float32
F32R = mybir.dt.float32r
BF16 = mybir.dt.bfloat16
U16 = mybir.dt.uint16
AF = mybir.ActivationFunctionType
ALU = mybir.AluOpType

B, C, H, W = 4, 128, 64, 64
N = H * W                    # 4096
NHALF = N // 2               # 2048 query positions per core
SCALE = 32 ** (-0.5)
MB = N // 128                # 32 key blocks of 128

# Schraudolph exp -> bf16 bits: exp(SCALE*s) ~ bitcast_u16(round(s*A + B))
A_SCH = SCALE * 128.0 / float(np.log(2.0))
B_SCH = 127.0 * 128.0 - 8.0

# wpack column-block indices (each block is 128 bf16 columns)
WQ0, WQ1, WK0, WK1, WV01, WPROJT, MS0 = 0, 1, 2, 3, 4, 5, 6
NWBLK = 15


def _alloc(tc):
    nc = tc.nc
    const = tc.alloc_tile_pool(name="const", bufs=1)
    epool = tc.alloc_tile_pool(name="epool", bufs=5)
    npool = tc.alloc_tile_pool(name="npool", bufs=2)
    psum = tc.alloc_tile_pool(name="psum", bufs=1, space="PSUM")
    pools = dict(const=const, epool=epool, npool=npool, psum=psum)

    t = {}
    specs = [
        ("wp", [128, NWBLK * 128], BF16), ("cst", [128, 4], F32),
        ("xr", [128, N], BF16),
        ("krep0", [128, N], BF16), ("krep1", [128, N], BF16),
        ("qrep0", [128, NHALF], BF16), ("qrep1", [128, NHALF], BF16),
        ("vchan", [128, N], BF16), ("vaugT", [128, MB * 130], BF16),
        ("vpad", [128, 34 * 66], BF16), ("attn", [128, NHALF], BF16),
        ("outsb", [128, NHALF], F32), ("ones64", [128, 64], F32R),
    ]
    for name, shape, dt in specs:
        t[name] = const.tile(shape, dt, name=name)
    return pools, t


def _emit_preamble(tc, t, io, pools):
    nc = tc.nc
    no_dma = os.environ.get("KPROBE", "") == "no_dma"
    if not no_dma:
        nc.sync.dma_start(out=t["wp"][:, 0:5 * 128], in_=io["wpack"][:, 0:5 * 128])
        nc.sync.dma_start(out=t["wp"][:, 5 * 128:NWBLK * 128],
                          in_=io["wpack"][:, 5 * 128:NWBLK * 128])
        nc.sync.dma_start(out=t["cst"], in_=io["cst"])
        for j in range(4):
            nc.sync.dma_start(out=t["xr"][:, j * 1024:(j + 1) * 1024],
                              in_=io["x_rot"][:, j * 1024:(j + 1) * 1024])

    one_col = t["cst"][:, 2:3]
    zero_col = t["cst"][:, 3:4]
    # Pin the exp table set before any ACT op picks a different one.
    actwarm = pools["npool"].tile([128, 1], F32, tag="actwarm", name="actwarm")
    nc.scalar.activation(out=actwarm, in_=one_col, func=AF.Exp, scale=1.0)
    # ones columns of the augmented-V^T stationary (cols 64,129 mod 130)
    vaug4 = t["vaugT"].rearrange("p (mb c) -> p mb c", mb=MB, c=130)
    nc.vector.tensor_copy(out=vaug4[:, :, 64:130:65],
                          in_=one_col.broadcast_to([128, MB, 2]))
    # zero left/right borders of the padded-V image (cols 0,65 mod 66)
    vpadr = t["vpad"].rearrange("p (r c) -> p r c", r=34, c=66)
    nc.vector.tensor_copy(out=vpadr[:, :, 0:66:65],
                          in_=zero_col.broadcast_to([128, 34, 2]))
    nc.vector.tensor_copy(out=t["ones64"], in_=one_col.broadcast_to([128, 64]))
    # first-iteration qkv chunks that gate the first S^T matmuls
    for blk, dst, j, off in [(WK0, t["krep0"], 0, 0), (WQ0, t["qrep0"], 0, 64),
                             (WQ0, t["qrep0"], 1, 64)]:
        pa = pools["psum"].tile([128, 512], F32, tag="s", bufs=3, name="pa")
        nc.tensor.matmul(pa, lhsT=t["wp"][:, blk * 128:(blk + 1) * 128],
                         rhs=t["xr"][:, off + j * 512:off + (j + 1) * 512],
                         start=True, stop=True)
        nc.vector.tensor_copy(out=dst[:, j * 512:(j + 1) * 512], in_=pa)


def _emit_body(tc, t, io, pools):
    nc = tc.nc
    psum, epool, npool = pools["psum"], pools["epool"], pools["npool"]
    probe = os.environ.get("KPROBE", "")
    kdve8 = int(os.environ.get("KDVE8", "2"))   # of 8 pair-tiles -> DVE exp
    no_dma = probe == "no_dma"
    skip_attn = probe == "no_attn"
    skip_oacc = probe in ("no_oacc", "no_exp")
    skip_exp = probe == "no_exp"

    wp, cst = t["wp"], t["cst"]
    xr, vchan, vaugT, vpad = t["xr"], t["vchan"], t["vaugT"], t["vpad"]
    attn, outsb, ones64 = t["attn"], t["outsb"], t["ones64"]

    def wblk(i):
        return wp[:, i * 128:(i + 1) * 128]

    # ---- phase A prologue: K0/Q0 so the attention stream can start -------
    def emit_pa(blk, src, dst, j, off=0):
        pa = psum.tile([128, 512], F32, tag="s", bufs=3, name="pa")
        nc.tensor.matmul(pa, lhsT=wblk(blk),
                         rhs=src[:, off + j * 512:off + (j + 1) * 512],
                         start=True, stop=True)
        nc.vector.tensor_copy(out=dst[:, j * 512:(j + 1) * 512], in_=pa)

    # The qkv conv is interleaved into the attention stream's engine slack.
    # The three chunks gating the first S^T matmuls (krep0 c0, qrep0 c0/c1)
    # are produced for the NEXT loop iteration at the tail of segment 3 (the
    # preamble provides them for the first iteration), so each iteration's
    # attention stream starts immediately.
    inj = {0: {}, 1: {}, 2: {}, 3: {}}
    for mb in range(7):
        inj[0][mb] = (WK0, xr, t["krep0"], mb + 1, 0)
    inj[0][24] = (WQ0, xr, t["qrep0"], 2, 64)
    inj[0][26] = (WQ0, xr, t["qrep0"], 3, 64)
    inj[1][0] = (WK1, xr, t["krep1"], 0, 0)
    inj[1][2] = (WQ1, xr, t["qrep1"], 0, 64)
    inj[1][4] = (WQ1, xr, t["qrep1"], 1, 64)
    for j in range(7):
        inj[2][2 * j + 1] = (WK1, xr, t["krep1"], j + 1, 0)
    inj[2][15] = (WQ1, xr, t["qrep1"], 2, 64)
    inj[2][17] = (WQ1, xr, t["qrep1"], 3, 64)
    for j in range(4):
        inj[2][19 + 2 * j] = (WV01, xr, vchan, j, 0)
    for j in range(4):
        inj[3][2 * j] = (WV01, xr, vchan, 4 + j, 0)
    inj[3][10] = (WK0, xr, t["krep0"], 0, 0)
    inj[3][12] = (WQ0, xr, t["qrep0"], 0, 64)
    inj[3][14] = (WQ0, xr, t["qrep0"], 1, 64)

    def emit_pt(mb):
        # V^T block straight from x: pt = x_blk^T @ Wv^T  -> [keys, dims]
        pt = psum.tile([128, 128], F32, tag="s", bufs=3, name="pt")
        nc.tensor.matmul(pt, lhsT=xr[:, mb * 128:(mb + 1) * 128],
                         rhs=wblk(WV01), start=True, stop=True)
        dst = vaugT[:, mb * 130:mb * 130 + 130].rearrange(
            "p (a b) -> p a b", a=2, b=65)[:, :, 0:64]
        src = pt.rearrange("p (a b) -> p a b", a=2, b=64)
        nc.vector.tensor_copy(out=dst, in_=src)

    vpad3 = vpad.rearrange("p (r c) -> p r c", r=34, c=66)

    if skip_attn or skip_oacc:
        # keep attn written for the probe builds (timing only)
        nc.vector.tensor_copy(
            out=attn, in_=cst[:, 3:4].broadcast_to([128, NHALF]))

    # ---- phase C: attention, one continuous S^T -> exp -> O stream -------
    # Four (head, query-half) segments share a single software pipeline so
    # the scalar engine never drains between segments.  The 32-deep S^T
    # contraction rotates the PE row-quadrant every matmul (rg=(2mb+cl)%4)
    # so consecutive S^T matmuls run concurrently on different row bands.
    # The kdve schedule sends part of each segment's exp() to the vector
    # engine (Schraudolph bit-trick); segment 0 keeps the vector engine
    # free for the qkv/V^T PSUM evacuations injected into its slack.
    LAG = int(os.environ.get("KLAG", "6"))
    kdve_sched = [0, kdve8, (3 * kdve8 + 1) // 2, (3 * kdve8 + 1) // 2]
    if os.environ.get("KDVE_SCHED"):
        kdve_sched = [int(v) for v in os.environ["KDVE_SCHED"].split(",")]
    segs = [] if skip_attn else [(h, p) for h in range(2) for p in range(2)]
    oaccs = {}
    pend = []

    def normalize(si):
        # rows 0:64 / row 64 (the ones-column accumulation)
        h, npass = segs[si]
        c0 = 2 * npass
        for cl in range(2):
            rec = npool.tile([128, 512], F32, tag="rec", name="rec")
            nc.vector.reciprocal(out=rec[64:65, :],
                                 in_=oaccs[si][cl][64:65, :])
            rb = npool.tile([128, 512], F32, tag="rb", name="rb")
            nc.gpsimd.partition_broadcast(out_ap=rb[0:64, :],
                                          in_=rec[64:65, :])
            nc.vector.tensor_mul(
                out=attn[h * 64:(h + 1) * 64,
                         (c0 + cl) * 512:(c0 + cl + 1) * 512],
                in0=oaccs[si][cl][0:64, :], in1=rb[0:64, :])

    def flush_o(si, mb, et):
        h = segs[si][0]
        for cl in range(2):
            nc.tensor.matmul(
                oaccs[si][cl][0:65, :],
                lhsT=vaugT[:, mb * 130 + h * 65:mb * 130 + h * 65 + 65],
                rhs=et[:, cl * 512:(cl + 1) * 512],
                start=(mb == 0), stop=(mb == MB - 1))
        if mb == MB - 1:
            normalize(si)

    for si, (h, npass) in enumerate(segs):
        krep = (t["krep0"], t["krep1"])[h]
        qrep = (t["qrep0"], t["qrep1"])[h]
        c0 = 2 * npass
        kdve = kdve_sched[si] if not skip_oacc else kdve8
        if not skip_oacc:
            oaccs[si] = [psum.tile([128, 512], F32, tag="o", bufs=2,
                                   name=f"oacc{cl}") for cl in range(2)]
        for mb in range(MB):
            # injected qkv / V^T work (phase A spread over the stream)
            if si == 0:
                emit_pt(mb)
            if mb in inj[si]:
                emit_pa(*inj[si][mb])
            if si == 3 and mb == 8:
                # vchan complete: padded-V image for the depthwise conv
                nc.vector.tensor_copy(out=vpad3[:, 1:33, 1:65],
                                      in_=vchan[:, 64:64 + 32 * 64])
                nc.vector.tensor_scalar(
                    out=vpad3[:, 0, 1:65], in0=vchan[:, 0:64],
                    scalar1=cst[:, 0:1], scalar2=None, op0=ALU.mult)
                nc.vector.tensor_scalar(
                    out=vpad3[:, 33, 1:65], in0=vchan[:, 33 * 64:34 * 64],
                    scalar1=cst[:, 1:2], scalar2=None, op0=ALU.mult)
            st = psum.tile([128, 1024], F32, tag="s", bufs=3, name="st")
            for cl in range(2):
                rg = 0 if os.environ.get("KRG0") else (2 * mb + cl) % 4
                nc.tensor.matmul(
                    st[:, cl * 512:(cl + 1) * 512],
                    lhsT=krep[32 * rg:32 * (rg + 1),
                              mb * 128:(mb + 1) * 128],
                    rhs=qrep[32 * rg:32 * (rg + 1),
                             (c0 + cl) * 512:(c0 + cl + 1) * 512],
                    start=True, stop=True,
                    tile_position=(32 * rg, 0))
            if skip_exp:
                continue
            et = epool.tile([128, 1024], BF16, tag="e", name="et")
            if ((mb + 1) * kdve) // 8 > (mb * kdve) // 8:
                with nc.allow_low_precision(reason="schraudolph exp"):
                    nc.vector.tensor_scalar(
                        out=et.bitcast(U16), in0=st,
                        scalar1=A_SCH, scalar2=B_SCH,
                        op0=ALU.mult, op1=ALU.add)
            else:
                nc.scalar.activation(out=et, in_=st, func=AF.Exp,
                                     scale=SCALE)
            if skip_oacc:
                continue
            pend.append((si, mb, et))
            if len(pend) > LAG:
                flush_o(*pend.pop(0))
        if si == 1 and not no_dma:
            # x / qkv weights reload; later readers get identical values
            nc.sync.dma_start(out=xr, in_=io["x_rot"])
            nc.sync.dma_start(out=wp[:, 0:5 * 128],
                              in_=io["wpack"][:, 0:5 * 128])
    for pe_ in pend:
        flush_o(*pe_)

    # ---- phase E: fused depthwise-conv + projection ----------------------
    shifts = [(dy, dx) for dy in (-1, 0, 1) for dx in (-1, 0, 1)]
    for cpair in range(2):
        psf = {}
        for cch in (2 * cpair, 2 * cpair + 1):
            psf[cch] = psum.tile([128, 512], F32, tag="s", bufs=3,
                                 name=f"psf{cch}")
        for widx in range(10):
            for cch in (2 * cpair, 2 * cpair + 1):
                if widx == 0:
                    lhsT = wblk(WPROJT)
                    rhs = attn[:, cch * 512:(cch + 1) * 512]
                else:
                    dy, dx = shifts[widx - 1]
                    lhsT = wblk(MS0 + widx - 1)
                    r0 = 1 + dy + 8 * cch
                    rhs = vpad3[:, r0:r0 + 8, 1 + dx:65 + dx]
                nc.tensor.matmul(psf[cch], lhsT=lhsT, rhs=rhs,
                                 start=(widx == 0), stop=(widx == 9))
        for cch in (2 * cpair, 2 * cpair + 1):
            sl = slice(cch * 512, (cch + 1) * 512)
            nc.vector.tensor_copy(out=outsb[:, sl], in_=psf[cch])
            if not no_dma:
                nc.sync.dma_start(out=io["out"][:, sl], in_=outsb[:, sl])

    # ---- reload proj/shift weights + consts for the next iteration -------
    if not no_dma:
        nc.sync.dma_start(out=wp[:, 5 * 128:NWBLK * 128],
                          in_=io["wpack"][:, 5 * 128:NWBLK * 128])
        nc.sync.dma_start(out=cst, in_=io["cst"])


def build_nc(reps=1):
    nc = bacc.Bacc(trn_type="TRN2", target_bir_lowering=False)
    io = {
        "wpack": nc.dram_tensor("wpack", [128, NWBLK * 128], BF16,
                                kind="ExternalInput").ap(),
        "cst": nc.dram_tensor("cst", [128, 4], F32,
                              kind="ExternalInput").ap(),
        "x_rot": nc.dram_tensor("x_rot", [128, N], BF16,
                                kind="ExternalInput").ap(),
        "out": nc.dram_tensor("out", [128, NHALF], F32,
                              kind="ExternalOutput").ap(),
    }
    with tile.TileContext(nc) as tc:
        pools, t = _alloc(tc)
        _emit_preamble(tc, t, io, pools)
        if reps == 1:
            _emit_body(tc, t, io, pools)
        else:
            with tc.For_i(0, reps, 1):
                _emit_body(tc, t, io, pools)
        for p in reversed(list(pools.values())):
            p.release()
    nc.compile()
    return nc


def host_prep(x, w_qkv, w_pe, w_proj):
    """Build the 8 per-core input maps from the full problem inputs."""
    bf16 = ml_dtypes.bfloat16
    x = np.ascontiguousarray(x, dtype=np.float32)
    wq = np.asarray(w_qkv, dtype=np.float32)[:, :, 0, 0]      # [256,128]
    wpe = np.asarray(w_pe, dtype=np.float32)[:, 0]            # [128,3,3]
    wpj = np.asarray(w_proj, dtype=np.float32)[:, :, 0, 0]    # [128,128]

    blocks = []
    for h in range(2):
        blocks.append(np.tile(wq[h * 128:h * 128 + 32], (4, 1)).T)       # WQh
    for h in range(2):
        blocks.append(np.tile(wq[h * 128 + 32:h * 128 + 64], (4, 1)).T)  # WKh
    blocks.insert(4, np.concatenate(
        [wq[64:128], wq[192:256]], axis=0).T)                 # WV01
    blocks.append(wpj.T)                                      # WPROJT
    for dy in (-1, 0, 1):
        for dx in (-1, 0, 1):
            blocks.append((wpj * wpe[:, dy + 1, dx + 1][None, :]).T)
    wpack = np.concatenate(blocks, axis=1).astype(bf16)       # [128, 15*128]

    in_maps = []
    for core in range(8):
        b, half = core // 2, core % 2
        y0 = 32 * half
        cst = np.zeros((128, 4), np.float32)
        cst[:, 0] = 1.0 if half == 1 else 0.0     # top halo valid?
        cst[:, 1] = 1.0 if half == 0 else 0.0     # bottom halo valid?
        cst[:, 2] = 1.0
        x_rot = np.roll(x[b], 1 - y0, axis=1).reshape(128, N).astype(bf16)
        in_maps.append({
            "wpack": np.ascontiguousarray(wpack),
            "cst": cst,
            "x_rot": np.ascontiguousarray(x_rot),
        })
    return in_maps


def assemble(results):
    out = np.zeros((B, C, H, W), np.float32)
    for core in range(8):
        b, half = core // 2, core % 2
        out[b, :, 32 * half:32 * half + 32, :] = \
            results[core]["out"].reshape(C, 32, W)
    return out


_NC_CACHE = {}


def _get_nc(reps=1):
    if reps not in _NC_CACHE:
        _NC_CACHE[reps] = build_nc(reps)
    return _NC_CACHE[reps]


def run(x, w_qkv, w_pe, w_proj, reps=1, **spmd_kwargs):
    nc = _get_nc(reps)
    in_maps = host_prep(x, w_qkv, w_pe, w_proj)
    res = run_bass_kernel_spmd(nc, in_maps, core_ids=list(range(8)),
                               **spmd_kwargs)
    return assemble(res.results), res


def kernel(x, w_qkv, w_pe, w_proj):
    out, _ = run(x, w_qkv, w_pe, w_proj)
    return out


# revision 31
# speedup vs baseline: 1.1507x; 1.0252x over previous
"""Trainium2 Bass kernel for nn_Attention_11192684774105.

Reference computation (B=4, C=128, H=W=64, N=4096, 2 heads, key_dim=32,
head_dim=64):
    qkv  = conv1x1(x, w_qkv)                    # [B,256,H,W]
    q,k,v split per head; attn = softmax(q^T k / sqrt(32)) over keys
    out  = v @ attn^T  (+ depthwise3x3(v, w_pe)) -> conv1x1(w_proj)

Sharding: 8 cores = (batch b, row-half) pairs.  Each core computes both
heads for 2048 query positions (32 image rows) of one sample; keys/values
span the full 4096 positions.  Output is a pure concatenation.

Per-core algorithm (bf16 compute, fp32 PSUM accumulation):
  - Inputs are pre-quantized to bf16 on the host (halves DMA traffic).
  - K/V come from a row-rotated copy of x so the depthwise-conv halo rows
    sit at fixed positions (attention is invariant to a consistent
    permutation of the key axis).
  - S^T = K^T Q is computed 512 queries per matmul with the 32-deep
    contraction placed on a rotating PE row-quadrant (tile_position);
    the rotation lets every weight load overlap the previous matmul
    (pinning one quadrant measures 2.2x slower).
  - exp() runs 1024 elements/op out of PSUM; most blocks on the scalar
    engine (|logits| < 3, no max subtraction), ~25% on the vector engine
    via a Schraudolph bit-trick straight into bf16 bits (the softmax
    ratio cancels most of the approximation error).  The denominator
    reciprocal row is broadcast across partitions with a ones-column
    matmul on the 64-row PE quadrant.
  - The softmax denominator comes from an extra ones-column in the V^T
    stationary operand of the second matmul (row 64 of the accumulator).
  - V^T blocks are produced directly as matmuls (x_blk^T @ Wv^T), not via
    PE transposes.
  - The depthwise conv + output projection are fused into 10 accumulating
    1x1 matmuls per query chunk at the stream tail.
  - Input DMA loads are software-pipelined: a preamble load before the
    timing loop, in-body reloads placed right after the last consumer.
    The qkv chunks gating the next iteration's first S^T matmuls are
    produced at the tail of segment 3, so under tc.For_i each iteration's
    attention stream starts immediately.
"""

import os

import numpy as np
import ml_dtypes

import concourse.bass as bass
import concourse.mybir as mybir
import concourse.tile as tile
from concourse import bacc
from concourse.bass_utils import run_bass_kernel_spmd

F32 = mybir.dt.float32
F32R = mybir.dt.float32r
BF16 = mybir.dt.bfloat16
U16 = mybir.dt.uint16
AF = mybir.ActivationFunctionType
ALU = mybir.AluOpType

B, C, H, W = 4, 128, 64, 64
N = H * W                    # 4096
NHALF = N // 2               # 2048 query positions per core
SCALE = 32 ** (-0.5)
MB = N // 128                # 32 key blocks of 128

# Schraudolph exp -> bf16 bits: exp(SCALE*s) ~ bitcast_u16(round(s*A + B))
A_SCH = SCALE * 128.0 / float(np.log(2.0))
B_SCH = 127.0 * 128.0 - 8.0

# wpack column-block indices (each block is 128 bf16 columns)
WQ0, WQ1, WK0, WK1, WV01, WPROJT, MS0 = 0, 1, 2, 3, 4, 5, 6
NWBLK = 15


def _alloc(tc):
    nc = tc.nc
    const = tc.alloc_tile_pool(name="const", bufs=1)
    epool = tc.alloc_tile_pool(name="epool", bufs=5)
    npool = tc.alloc_tile_pool(name="npool", bufs=2)
    psum = tc.alloc_tile_pool(name="psum", bufs=1, space="PSUM")
    pools = dict(const=const, epool=epool, npool=npool, psum=psum)

    t = {}
    specs = [
        ("wp", [128, NWBLK * 128], BF16), ("cst", [128, 4], F32),
        ("xr", [128, N], BF16),
        ("krep0", [128, N], BF16), ("krep1", [128, N], BF16),
        ("qrep0", [128, NHALF], BF16), ("qrep1", [128, NHALF], BF16),
        ("vchan", [128, N], BF16), ("vaugT", [128, MB * 130], BF16),
        ("vpad", [128, 34 * 66], BF16), ("attn", [128, NHALF], BF16),
        ("outsb", [128, NHALF], F32), ("ones64", [128, 64], F32R),
    ]
    for name, shape, dt in specs:
        t[name] = const.tile(shape, dt, name=name)
    return pools, t


def _emit_preamble(tc, t, io, pools):
    nc = tc.nc
    no_dma = os.environ.get("KPROBE", "") == "no_dma"
    if not no_dma:
        nc.sync.dma_start(out=t["wp"][:, 0:5 * 128], in_=io["wpack"][:, 0:5 * 128])
        nc.sync.dma_start(out=t["wp"][:, 5 * 128:NWBLK * 128],
                          in_=io["wpack"][:, 5 * 128:NWBLK * 128])
        nc.sync.dma_start(out=t["cst"], in_=io["cst"])
        for j in range(4):
            nc.sync.dma_start(out=t["xr"][:, j * 1024:(j + 1) * 1024],
                              in_=io["x_rot"][:, j * 1024:(j + 1) * 1024])

    one_col = t["cst"][:, 2:3]
    zero_col = t["cst"][:, 3:4]
    # Pin the exp table set before any ACT op picks a different one.
    actwarm = pools["npool"].tile([128, 1], F32, tag="actwarm", name="actwarm")
    nc.scalar.activation(out=actwarm, in_=one_col, func=AF.Exp, scale=1.0)
    # ones columns of the augmented-V^T stationary (cols 64,129 mod 130)
    vaug4 = t["vaugT"].rearrange("p (mb c) -> p mb c", mb=MB, c=130)
    nc.vector.tensor_copy(out=vaug4[:, :, 64:130:65],
                          in_=one_col.broadcast_to([128, MB, 2]))
    # zero left/right borders of the padded-V image (cols 0,65 mod 66)
    vpadr = t["vpad"].rearrange("p (r c) -> p r c", r=34, c=66)
    nc.vector.tensor_copy(out=vpadr[:, :, 0:66:65],
                          in_=zero_col.broadcast_to([128, 34, 2]))
    nc.vector.tensor_copy(out=t["ones64"], in_=one_col.broadcast_to([128, 64]))
    # first-iteration qkv chunks that gate the first S^T matmuls
    for blk, dst, j, off in [(WK0, t["krep0"], 0, 0), (WQ0, t["qrep0"], 0, 64),
                             (WQ0, t["qrep0"], 1, 64)]:
        pa = pools["psum"].tile([128, 512], F32, tag="s", bufs=3, name="pa")
        nc.tensor.matmul(pa, lhsT=t["wp"][:, blk * 128:(blk + 1) * 128],
                         rhs=t["xr"][:, off + j * 512:off + (j + 1) * 512],
                         start=True, stop=True)
        nc.vector.tensor_copy(out=dst[:, j * 512:(j + 1) * 512], in_=pa)


def _emit_body(tc, t, io, pools):
    nc = tc.nc
    psum, epool, npool = pools["psum"], pools["epool"], pools["npool"]
    probe = os.environ.get("KPROBE", "")
    kdve8 = int(os.environ.get("KDVE8", "2"))   # of 8 pair-tiles -> DVE exp
    no_dma = probe == "no_dma"
    skip_attn = probe == "no_attn"
    skip_oacc = probe in ("no_oacc", "no_exp")
    skip_exp = probe == "no_exp"

    wp, cst = t["wp"], t["cst"]
    xr, vchan, vaugT, vpad = t["xr"], t["vchan"], t["vaugT"], t["vpad"]
    attn, outsb, ones64 = t["attn"], t["outsb"], t["ones64"]

    def wblk(i):
        return wp[:, i * 128:(i + 1) * 128]

    # ---- phase A prologue: K0/Q0 so the attention stream can start -------
    def emit_pa(blk, src, dst, j, off=0):
        pa = psum.tile([128, 512], F32, tag="s", bufs=3, name="pa")
        nc.tensor.matmul(pa, lhsT=wblk(blk),
                         rhs=src[:, off + j * 512:off + (j + 1) * 512],
                         start=True, stop=True)
        nc.vector.tensor_copy(out=dst[:, j * 512:(j + 1) * 512], in_=pa)

    # The qkv conv is interleaved into the attention stream's engine slack.
    # The three chunks gating the first S^T matmuls (krep0 c0, qrep0 c0/c1)
    # are produced for the NEXT loop iteration at the tail of segment 3 (the
    # preamble provides them for the first iteration), so each iteration's
    # attention stream starts immediately.
    inj = {0: {}, 1: {}, 2: {}, 3: {}}
    for mb in range(7):
        inj[0][mb] = (WK0, xr, t["krep0"], mb + 1, 0)
    inj[0][24] = (WQ0, xr, t["qrep0"], 2, 64)
    inj[0][26] = (WQ0, xr, t["qrep0"], 3, 64)
    inj[1][0] = (WK1, xr, t["krep1"], 0, 0)
    inj[1][2] = (WQ1, xr, t["qrep1"], 0, 64)
    inj[1][4] = (WQ1, xr, t["qrep1"], 1, 64)
    for j in range(7):
        inj[2][2 * j + 1] = (WK1, xr, t["krep1"], j + 1, 0)
    inj[2][15] = (WQ1, xr, t["qrep1"], 2, 64)
    inj[2][17] = (WQ1, xr, t["qrep1"], 3, 64)
    for j in range(4):
        inj[2][19 + 2 * j] = (WV01, xr, vchan, j, 0)
    for j in range(4):
        inj[3][2 * j] = (WV01, xr, vchan, 4 + j, 0)
    inj[3][10] = (WK0, xr, t["krep0"], 0, 0)
    inj[3][12] = (WQ0, xr, t["qrep0"], 0, 64)
    inj[3][14] = (WQ0, xr, t["qrep0"], 1, 64)

    def emit_pt(mb):
        # V^T block straight from x: pt = x_blk^T @ Wv^T  -> [keys, dims]
        pt = psum.tile([128, 128], F32, tag="s", bufs=3, name="pt")
        nc.tensor.matmul(pt, lhsT=xr[:, mb * 128:(mb + 1) * 128],
                         rhs=wblk(WV01), start=True, stop=True)
        dst = vaugT[:, mb * 130:mb * 130 + 130].rearrange(
            "p (a b) -> p a b", a=2, b=65)[:, :, 0:64]
        src = pt.rearrange("p (a b) -> p a b", a=2, b=64)
        nc.vector.tensor_copy(out=dst, in_=src)

    vpad3 = vpad.rearrange("p (r c) -> p r c", r=34, c=66)

    if skip_attn or skip_oacc:
        # keep attn written for the probe builds (timing only)
        nc.vector.tensor_copy(
            out=attn, in_=cst[:, 3:4].broadcast_to([128, NHALF]))

    # ---- phase C: attention, one continuous S^T -> exp -> O stream -------
    # Four (head, query-half) segments share a single software pipeline so
    # the scalar engine never drains between segments.  The 32-deep S^T
    # contraction rotates the PE row-quadrant every matmul (rg=(2mb+cl)%4)
    # so consecutive S^T matmuls run concurrently on different row bands.
    # The kdve schedule sends part of each segment's exp() to the vector
    # engine (Schraudolph bit-trick); segment 0 keeps the vector engine
    # free for the qkv/V^T PSUM evacuations injected into its slack.
    LAG = int(os.environ.get("KLAG", "6"))
    kdve_sched = [0, kdve8, (3 * kdve8 + 1) // 2, (3 * kdve8 + 1) // 2]
    if os.environ.get("KDVE_SCHED"):
        kdve_sched = [int(v) for v in os.environ["KDVE_SCHED"].split(",")]
    segs = [] if skip_attn else [(h, p) for h in range(2) for p in range(2)]
    oaccs = {}
    pend = []

    def normalize(si):
        # rows 0:64 / row 64 (the ones-column accumulation)
        h, npass = segs[si]
        c0 = 2 * npass
        for cl in range(2):
            rec = npool.tile([128, 512], F32, tag="rec", name="rec")
            nc.vector.reciprocal(out=rec[64:65, :],
                                 in_=oaccs[si][cl][64:65, :])
            rb = npool.tile([128, 512], F32, tag="rb", name="rb")
            nc.gpsimd.partition_broadcast(out_ap=rb[0:64, :],
                                          in_=rec[64:65, :])
            nc.vector.tensor_mul(
                out=attn[h * 64:(h + 1) * 64,
                         (c0 + cl) * 512:(c0 + cl + 1) * 512],
                in0=oaccs[si][cl][0:64, :], in1=rb[0:64, :])

    def flush_o(si, mb, et):
        h = segs[si][0]
        for cl in range(2):
            nc.tensor.matmul(
                oaccs[si][cl][0:65, :],
                lhsT=vaugT[:, mb * 130 + h * 65:mb * 130 + h * 65 + 65],
                rhs=et[:, cl * 512:(cl + 1) * 512],
                start=(mb == 0), stop=(mb == MB - 1))
        if mb == MB - 1:
            normalize(si)

    for si, (h, npass) in enumerate(segs):
        krep = (t["krep0"], t["krep1"])[h]
        qrep = (t["qrep0"], t["qrep1"])[h]
        c0 = 2 * npass
        kdve = kdve_sched[si] if not skip_oacc else kdve8
        if not skip_oacc:
            oaccs[si] = [psum.tile([128, 512], F32, tag="o", bufs=2,
                                   name=f"oacc{cl}") for cl in range(2)]
        for mb in range(MB):
            # injected qkv / V^T work (phase A spread over the stream)
            if si == 0:
                emit_pt(mb)
            if mb in inj[si]:
                emit_pa(*inj[si][mb])
            if si == 3 and mb == 8:
                # vchan complete: padded-V image for the depthwise conv
                nc.vector.tensor_copy(out=vpad3[:, 1:33, 1:65],
                                      in_=vchan[:, 64:64 + 32 * 64])
                nc.vector.tensor_scalar(
                    out=vpad3[:, 0, 1:65], in0=vchan[:, 0:64],
                    scalar1=cst[:, 0:1], scalar2=None, op0=ALU.mult)
                nc.vector.tensor_scalar(
                    out=vpad3[:, 33, 1:65], in0=vchan[:, 33 * 64:34 * 64],
                    scalar1=cst[:, 1:2], scalar2=None, op0=ALU.mult)
            st = psum.tile([128, 1024], F32, tag="s", bufs=3, name="st")
            for cl in range(2):
                rg = 0 if os.environ.get("KRG0") else (2 * mb + cl) % 4
                nc.tensor.matmul(
                    st[:, cl * 512:(cl + 1) * 512],
                    lhsT=krep[32 * rg:32 * (rg + 1),
                              mb * 128:(mb + 1) * 128],
                    rhs=qrep[32 * rg:32 * (rg + 1),
                             (c0 + cl) * 512:(c0 + cl + 1) * 512],
                    start=True, stop=True,
                    tile_position=(32 * rg, 0))
            if skip_exp:
                continue
            et = epool.tile([128, 1024], BF16, tag="e", name="et")
            if ((mb + 1) * kdve) // 8 > (mb * kdve) // 8:
                with nc.allow_low_precision(reason="schraudolph exp"):
                    nc.vector.tensor_scalar(
                        out=et.bitcast(U16), in0=st,
                        scalar1=A_SCH, scalar2=B_SCH,
                        op0=ALU.mult, op1=ALU.add)
            else:
                nc.scalar.activation(out=et, in_=st, func=AF.Exp,
                                     scale=SCALE)
            if skip_oacc:
                continue
            pend.append((si, mb, et))
            lag_eff = LAG
            if si == 3 and mb > 25:
                lag_eff = max(1, LAG - (mb - 25))
            while len(pend) > lag_eff:
                flush_o(*pend.pop(0))
        if si == 1 and not no_dma:
            # x / qkv weights reload; later readers get identical values
            nc.sync.dma_start(out=xr, in_=io["x_rot"])
            nc.sync.dma_start(out=wp[:, 0:5 * 128],
                              in_=io["wpack"][:, 0:5 * 128])
    for pe_ in pend:
        flush_o(*pe_)

    # ---- phase E: fused depthwise-conv + projection ----------------------
    shifts = [(dy, dx) for dy in (-1, 0, 1) for dx in (-1, 0, 1)]
    for cpair in range(2):
        psf = {}
        for cch in (2 * cpair, 2 * cpair + 1):
            psf[cch] = psum.tile([128, 512], F32, tag="s", bufs=3,
                                 name=f"psf{cch}")
        for widx in range(10):
            for cch in (2 * cpair, 2 * cpair + 1):
                if widx == 0:
                    lhsT = wblk(WPROJT)
                    rhs = attn[:, cch * 512:(cch + 1) * 512]
                else:
                    dy, dx = shifts[widx - 1]
                    lhsT = wblk(MS0 + widx - 1)
                    r0 = 1 + dy + 8 * cch
                    rhs = vpad3[:, r0:r0 + 8, 1 + dx:65 + dx]
                nc.tensor.matmul(psf[cch], lhsT=lhsT, rhs=rhs,
                                 start=(widx == 0), stop=(widx == 9))
        for idx, cch in enumerate((2 * cpair, 2 * cpair + 1)):
            sl = slice(cch * 512, (cch + 1) * 512)
            if idx:
                nc.scalar.copy(out=outsb[:, sl], in_=psf[cch])
            else:
                nc.vector.tensor_copy(out=outsb[:, sl], in_=psf[cch])
            if not no_dma:
                nc.sync.dma_start(out=io["out"][:, sl], in_=outsb[:, sl])

    # ---- reload proj/shift weights + consts for the next iteration -------
    if not no_dma:
        nc.sync.dma_start(out=wp[:, 5 * 128:NWBLK * 128],
                          in_=io["wpack"][:, 5 * 128:NWBLK * 128])
        nc.sync.dma_start(out=cst, in_=io["cst"])


def build_nc(reps=1):
    nc = bacc.Bacc(trn_type="TRN2", target_bir_lowering=False)
    io = {
        "wpack": nc.dram_tensor("wpack", [128, NWBLK * 128], BF16,
                                kind="ExternalInput").ap(),
        "cst": nc.dram_tensor("cst", [128, 4], F32,
                              kind="ExternalInput").ap(),
        "x_rot": nc.dram_tensor("x_rot", [128, N], BF16,
                                kind="ExternalInput").ap(),
        "out": nc.dram_tensor("out", [128, NHALF], F32,
                              kind="ExternalOutput").ap(),
    }
    with tile.TileContext(nc) as tc:
        pools, t = _alloc(tc)
        _emit_preamble(tc, t, io, pools)
        if reps == 1:
            _emit_body(tc, t, io, pools)
        else:
            with tc.For_i(0, reps, 1):
                _emit_body(tc, t, io, pools)
        for p in reversed(list(pools.values())):
            p.release()
    nc.compile()
    return nc


def host_prep(x, w_qkv, w_pe, w_proj):
    """Build the 8 per-core input maps from the full problem inputs."""
    bf16 = ml_dtypes.bfloat16
    x = np.ascontiguousarray(x, dtype=np.float32)
    wq = np.asarray(w_qkv, dtype=np.float32)[:, :, 0, 0]      # [256,128]
    wpe = np.asarray(w_pe, dtype=np.float32)[:, 0]            # [128,3,3]
    wpj = np.asarray(w_proj, dtype=np.float32)[:, :, 0, 0]    # [128,128]

    blocks = []
    for h in range(2):
        blocks.append(np.tile(wq[h * 128:h * 128 + 32], (4, 1)).T)       # WQh
    for h in range(2):
        blocks.append(np.tile(wq[h * 128 + 32:h * 128 + 64], (4, 1)).T)  # WKh
    blocks.insert(4, np.concatenate(
        [wq[64:128], wq[192:256]], axis=0).T)                 # WV01
    blocks.append(wpj.T)                                      # WPROJT
    for dy in (-1, 0, 1):
        for dx in (-1, 0, 1):
            blocks.append((wpj * wpe[:, dy + 1, dx + 1][None, :]).T)
    wpack = np.concatenate(blocks, axis=1).astype(bf16)       # [128, 15*128]

    in_maps = []
    for core in range(8):
        b, half = core // 2, core % 2
        y0 = 32 * half
        cst = np.zeros((128, 4), np.float32)
        cst[:, 0] = 1.0 if half == 1 else 0.0     # top halo valid?
        cst[:, 1] = 1.0 if half == 0 else 0.0     # bottom halo valid?
        cst[:, 2] = 1.0
        x_rot = np.roll(x[b], 1 - y0, axis=1).reshape(128, N).astype(bf16)
        in_maps.append({
            "wpack": np.ascontiguousarray(wpack),
            "cst": cst,
            "x_rot": np.ascontiguousarray(x_rot),
        })
    return in_maps


def assemble(results):
    out = np.zeros((B, C, H, W), np.float32)
    for core in range(8):
        b, half = core // 2, core % 2
        out[b, :, 32 * half:32 * half + 32, :] = \
            results[core]["out"].reshape(C, 32, W)
    return out


_NC_CACHE = {}


def _get_nc(reps=1):
    if reps not in _NC_CACHE:
        _NC_CACHE[reps] = build_nc(reps)
    return _NC_CACHE[reps]


def run(x, w_qkv, w_pe, w_proj, reps=1, **spmd_kwargs):
    nc = _get_nc(reps)
    in_maps = host_prep(x, w_qkv, w_pe, w_proj)
    res = run_bass_kernel_spmd(nc, in_maps, core_ids=list(range(8)),
                               **spmd_kwargs)
    return assemble(res.results), res


def kernel(x, w_qkv, w_pe, w_proj):
    out, _ = run(x, w_qkv, w_pe, w_proj)
    return out


# revision 32
# speedup vs baseline: 1.2402x; 1.0778x over previous
"""Trainium2 Bass kernel for nn_Attention_11192684774105.

Reference computation (B=4, C=128, H=W=64, N=4096, 2 heads, key_dim=32,
head_dim=64):
    qkv  = conv1x1(x, w_qkv)                    # [B,256,H,W]
    q,k,v split per head; attn = softmax(q^T k / sqrt(32)) over keys
    out  = v @ attn^T  (+ depthwise3x3(v, w_pe)) -> conv1x1(w_proj)

Sharding: 8 cores = (batch b, row-half) pairs.  Each core computes both
heads for 2048 query positions (32 image rows) of one sample; keys/values
span the full 4096 positions.  Output is a pure concatenation.

Per-core algorithm (bf16 compute, fp32 PSUM accumulation):
  - Inputs are pre-quantized to bf16 on the host (halves DMA traffic).
  - K/V come from a row-rotated copy of x so the depthwise-conv halo rows
    sit at fixed positions (attention is invariant to a consistent
    permutation of the key axis).
  - S^T = K^T Q is computed 512 queries per matmul with the 32-deep
    contraction placed on a rotating PE row-quadrant (tile_position);
    the rotation lets every weight load overlap the previous matmul
    (pinning one quadrant measures 2.2x slower).
  - exp() runs 1024 elements/op out of PSUM; most blocks on the scalar
    engine (|logits| < 3, no max subtraction), ~25% on the vector engine
    via a Schraudolph bit-trick straight into bf16 bits (the softmax
    ratio cancels most of the approximation error).  The denominator
    reciprocal row is broadcast across partitions with a ones-column
    matmul on the 64-row PE quadrant.
  - The softmax denominator comes from an extra ones-column in the V^T
    stationary operand of the second matmul (row 64 of the accumulator).
  - V^T blocks are produced directly as matmuls (x_blk^T @ Wv^T), not via
    PE transposes.
  - The depthwise conv + output projection are fused into 10 accumulating
    1x1 matmuls per query chunk at the stream tail.
  - Input DMA loads are software-pipelined: a preamble load before the
    timing loop, in-body reloads placed right after the last consumer.
    The qkv chunks gating the next iteration's first S^T matmuls are
    produced at the tail of segment 3, so under tc.For_i each iteration's
    attention stream starts immediately.
"""

import os

import numpy as np
import ml_dtypes

import concourse.bass as bass
import concourse.mybir as mybir
import concourse.tile as tile
from concourse import bacc
from concourse.bass_utils import run_bass_kernel_spmd

F32 = mybir.dt.float32
F32R = mybir.dt.float32r
BF16 = mybir.dt.bfloat16
U16 = mybir.dt.uint16
AF = mybir.ActivationFunctionType
ALU = mybir.AluOpType

B, C, H, W = 4, 128, 64, 64
N = H * W                    # 4096
NHALF = N // 2               # 2048 query positions per core
SCALE = 32 ** (-0.5)
MB = N // 128                # 32 key blocks of 128

# Schraudolph exp -> bf16 bits: exp(SCALE*s) ~ bitcast_u16(round(s*A + B))
A_SCH = SCALE * 128.0 / float(np.log(2.0))
B_SCH = 127.0 * 128.0 - 8.0

# wpack column-block indices (each block is 128 bf16 columns)
WQ0, WQ1, WK0, WK1, WV01, WPROJT, MS0 = 0, 1, 2, 3, 4, 5, 6
NWBLK = 15


def _alloc(tc):
    nc = tc.nc
    const = tc.alloc_tile_pool(name="const", bufs=1)
    epool = tc.alloc_tile_pool(name="epool", bufs=5)
    npool = tc.alloc_tile_pool(name="npool", bufs=2)
    psum = tc.alloc_tile_pool(name="psum", bufs=1, space="PSUM")
    pools = dict(const=const, epool=epool, npool=npool, psum=psum)

    t = {}
    specs = [
        ("wp", [128, NWBLK * 128], BF16), ("cst", [128, 4], F32),
        ("xr", [128, N], BF16),
        ("krep0", [128, N], BF16), ("krep1", [128, N], BF16),
        ("qrep0", [128, NHALF], BF16), ("qrep1", [128, NHALF], BF16),
        ("vchan", [128, N], BF16), ("vaugT", [128, MB * 130], BF16),
        ("vpad", [128, 34 * 66], BF16), ("attn", [128, NHALF], BF16),
        ("outsb", [128, NHALF], F32), ("ones64", [128, 64], F32R),
    ]
    for name, shape, dt in specs:
        t[name] = const.tile(shape, dt, name=name)
    return pools, t


def _emit_preamble(tc, t, io, pools):
    nc = tc.nc
    no_dma = os.environ.get("KPROBE", "") == "no_dma"
    if not no_dma:
        nc.sync.dma_start(out=t["wp"][:, 0:5 * 128], in_=io["wpack"][:, 0:5 * 128])
        nc.sync.dma_start(out=t["wp"][:, 5 * 128:NWBLK * 128],
                          in_=io["wpack"][:, 5 * 128:NWBLK * 128])
        nc.sync.dma_start(out=t["cst"], in_=io["cst"])
        for j in range(4):
            nc.sync.dma_start(out=t["xr"][:, j * 1024:(j + 1) * 1024],
                              in_=io["x_rot"][:, j * 1024:(j + 1) * 1024])

    one_col = t["cst"][:, 2:3]
    zero_col = t["cst"][:, 3:4]
    # Pin the exp table set before any ACT op picks a different one.
    actwarm = pools["npool"].tile([128, 1], F32, tag="actwarm", name="actwarm")
    nc.scalar.activation(out=actwarm, in_=one_col, func=AF.Exp, scale=1.0)
    # ones columns of the augmented-V^T stationary (cols 64,129 mod 130)
    vaug4 = t["vaugT"].rearrange("p (mb c) -> p mb c", mb=MB, c=130)
    nc.vector.tensor_copy(out=vaug4[:, :, 64:130:65],
                          in_=one_col.broadcast_to([128, MB, 2]))
    # zero left/right borders of the padded-V image (cols 0,65 mod 66)
    vpadr = t["vpad"].rearrange("p (r c) -> p r c", r=34, c=66)
    nc.vector.tensor_copy(out=vpadr[:, :, 0:66:65],
                          in_=zero_col.broadcast_to([128, 34, 2]))
    nc.vector.tensor_copy(out=t["ones64"], in_=one_col.broadcast_to([128, 64]))
    # first-iteration qkv chunks that gate the first S^T matmuls
    for blk, dst, j, off in [(WK0, t["krep0"], 0, 0), (WQ0, t["qrep0"], 0, 64),
                             (WQ0, t["qrep0"], 1, 64)]:
        pa = pools["psum"].tile([128, 512], F32, tag="s", bufs=3, name="pa")
        nc.tensor.matmul(pa, lhsT=t["wp"][:, blk * 128:(blk + 1) * 128],
                         rhs=t["xr"][:, off + j * 512:off + (j + 1) * 512],
                         start=True, stop=True)
        nc.vector.tensor_copy(out=dst[:, j * 512:(j + 1) * 512], in_=pa)


def _emit_body(tc, t, io, pools):
    nc = tc.nc
    psum, epool, npool = pools["psum"], pools["epool"], pools["npool"]
    probe = os.environ.get("KPROBE", "")
    kdve8 = int(os.environ.get("KDVE8", "2"))   # of 8 pair-tiles -> DVE exp
    no_dma = probe == "no_dma"
    skip_attn = probe == "no_attn"
    skip_oacc = probe in ("no_oacc", "no_exp")
    skip_exp = probe == "no_exp"

    wp, cst = t["wp"], t["cst"]
    xr, vchan, vaugT, vpad = t["xr"], t["vchan"], t["vaugT"], t["vpad"]
    attn, outsb, ones64 = t["attn"], t["outsb"], t["ones64"]

    def wblk(i):
        return wp[:, i * 128:(i + 1) * 128]

    # ---- phase A prologue: K0/Q0 so the attention stream can start -------
    def emit_pa(blk, src, dst, j, off=0):
        pa = psum.tile([128, 512], F32, tag="s", bufs=3, name="pa")
        nc.tensor.matmul(pa, lhsT=wblk(blk),
                         rhs=src[:, off + j * 512:off + (j + 1) * 512],
                         start=True, stop=True)
        nc.vector.tensor_copy(out=dst[:, j * 512:(j + 1) * 512], in_=pa)

    # The qkv conv is interleaved into the attention stream's engine slack.
    # The three chunks gating the first S^T matmuls (krep0 c0, qrep0 c0/c1)
    # are produced for the NEXT loop iteration at the tail of segment 3 (the
    # preamble provides them for the first iteration), so each iteration's
    # attention stream starts immediately.
    inj = {0: {}, 1: {}, 2: {}, 3: {}}
    for mb in range(7):
        inj[0][mb] = (WK0, xr, t["krep0"], mb + 1, 0)
    inj[0][24] = (WQ0, xr, t["qrep0"], 2, 64)
    inj[0][26] = (WQ0, xr, t["qrep0"], 3, 64)
    inj[1][0] = (WK1, xr, t["krep1"], 0, 0)
    inj[1][2] = (WQ1, xr, t["qrep1"], 0, 64)
    inj[1][4] = (WQ1, xr, t["qrep1"], 1, 64)
    for j in range(7):
        inj[2][2 * j + 1] = (WK1, xr, t["krep1"], j + 1, 0)
    inj[2][15] = (WQ1, xr, t["qrep1"], 2, 64)
    inj[2][17] = (WQ1, xr, t["qrep1"], 3, 64)
    for j in range(4):
        inj[2][19 + 2 * j] = (WV01, xr, vchan, j, 0)
    for j in range(4):
        inj[3][2 * j] = (WV01, xr, vchan, 4 + j, 0)
    inj[3][10] = (WK0, xr, t["krep0"], 0, 0)
    inj[3][12] = (WQ0, xr, t["qrep0"], 0, 64)
    inj[3][14] = (WQ0, xr, t["qrep0"], 1, 64)

    def emit_pt(mb):
        # V^T block straight from x: pt = x_blk^T @ Wv^T  -> [keys, dims]
        pt = psum.tile([128, 128], F32, tag="s", bufs=3, name="pt")
        nc.tensor.matmul(pt, lhsT=xr[:, mb * 128:(mb + 1) * 128],
                         rhs=wblk(WV01), start=True, stop=True)
        dst = vaugT[:, mb * 130:mb * 130 + 130].rearrange(
            "p (a b) -> p a b", a=2, b=65)[:, :, 0:64]
        src = pt.rearrange("p (a b) -> p a b", a=2, b=64)
        nc.vector.tensor_copy(out=dst, in_=src)

    vpad3 = vpad.rearrange("p (r c) -> p r c", r=34, c=66)

    if skip_attn or skip_oacc:
        # keep attn written for the probe builds (timing only)
        nc.vector.tensor_copy(
            out=attn, in_=cst[:, 3:4].broadcast_to([128, NHALF]))

    # ---- phase C: attention, one continuous S^T -> exp -> O stream -------
    # Four (head, query-half) segments share a single software pipeline so
    # the scalar engine never drains between segments.  The 32-deep S^T
    # contraction rotates the PE row-quadrant every matmul (rg=(2mb+cl)%4)
    # so consecutive S^T matmuls run concurrently on different row bands.
    # The kdve schedule sends part of each segment's exp() to the vector
    # engine (Schraudolph bit-trick); segment 0 keeps the vector engine
    # free for the qkv/V^T PSUM evacuations injected into its slack.
    LAG = int(os.environ.get("KLAG", "6"))
    kdve_sched = ([0, 3, 3, 3] if kdve8 == 2 else
                  [0, kdve8, (3 * kdve8 + 1) // 2, (3 * kdve8 + 1) // 2])
    if os.environ.get("KDVE_SCHED"):
        kdve_sched = [int(v) for v in os.environ["KDVE_SCHED"].split(",")]
    segs = [] if skip_attn else [(h, p) for h in range(2) for p in range(2)]
    oaccs = {}
    pend = []

    def normalize(si):
        # rows 0:64 / row 64 (the ones-column accumulation)
        h, npass = segs[si]
        c0 = 2 * npass
        for cl in range(2):
            rec = npool.tile([128, 512], F32, tag="rec", name="rec")
            nc.vector.reciprocal(out=rec[64:65, :],
                                 in_=oaccs[si][cl][64:65, :])
            rb = npool.tile([128, 512], F32, tag="rb", name="rb")
            nc.gpsimd.partition_broadcast(out_ap=rb[0:64, :],
                                          in_=rec[64:65, :])
            nc.vector.tensor_mul(
                out=attn[h * 64:(h + 1) * 64,
                         (c0 + cl) * 512:(c0 + cl + 1) * 512],
                in0=oaccs[si][cl][0:64, :], in1=rb[0:64, :])

    def flush_o(si, mb, et):
        h = segs[si][0]
        for cl in range(2):
            nc.tensor.matmul(
                oaccs[si][cl][0:65, :],
                lhsT=vaugT[:, mb * 130 + h * 65:mb * 130 + h * 65 + 65],
                rhs=et[:, cl * 512:(cl + 1) * 512],
                start=(mb == 0), stop=(mb == MB - 1))
        if mb == MB - 1:
            normalize(si)

    for si, (h, npass) in enumerate(segs):
        krep = (t["krep0"], t["krep1"])[h]
        qrep = (t["qrep0"], t["qrep1"])[h]
        c0 = 2 * npass
        kdve = kdve_sched[si] if not skip_oacc else kdve8
        if not skip_oacc:
            oaccs[si] = [psum.tile([128, 512], F32, tag="o", bufs=2,
                                   name=f"oacc{cl}") for cl in range(2)]
        for mb in range(MB):
            # injected qkv / V^T work (phase A spread over the stream)
            if si == 0:
                emit_pt(mb)
            if mb in inj[si]:
                emit_pa(*inj[si][mb])
            if si == 3 and mb == 8:
                # vchan complete: padded-V image for the depthwise conv
                nc.vector.tensor_copy(out=vpad3[:, 1:33, 1:65],
                                      in_=vchan[:, 64:64 + 32 * 64])
                nc.vector.tensor_scalar(
                    out=vpad3[:, 0, 1:65], in0=vchan[:, 0:64],
                    scalar1=cst[:, 0:1], scalar2=None, op0=ALU.mult)
                nc.vector.tensor_scalar(
                    out=vpad3[:, 33, 1:65], in0=vchan[:, 33 * 64:34 * 64],
                    scalar1=cst[:, 1:2], scalar2=None, op0=ALU.mult)
            st = psum.tile([128, 1024], F32, tag="s", bufs=3, name="st")
            for cl in range(2):
                rg = 0 if os.environ.get("KRG0") else (2 * mb + cl) % 4
                nc.tensor.matmul(
                    st[:, cl * 512:(cl + 1) * 512],
                    lhsT=krep[32 * rg:32 * (rg + 1),
                              mb * 128:(mb + 1) * 128],
                    rhs=qrep[32 * rg:32 * (rg + 1),
                             (c0 + cl) * 512:(c0 + cl + 1) * 512],
                    start=True, stop=True,
                    tile_position=(32 * rg, 0))
            if skip_exp:
                continue
            et = epool.tile([128, 1024], BF16, tag="e", name="et")
            if ((mb + 1) * kdve) // 8 > (mb * kdve) // 8:
                with nc.allow_low_precision(reason="schraudolph exp"):
                    nc.vector.tensor_scalar(
                        out=et.bitcast(U16), in0=st,
                        scalar1=A_SCH, scalar2=B_SCH,
                        op0=ALU.mult, op1=ALU.add)
            else:
                nc.scalar.activation(out=et, in_=st, func=AF.Exp,
                                     scale=SCALE)
            if skip_oacc:
                continue
            pend.append((si, mb, et))
            lag_eff = LAG
            if si == 3 and mb > 25:
                lag_eff = max(1, LAG - (mb - 25))
            while len(pend) > lag_eff:
                flush_o(*pend.pop(0))
        if si == 1 and not no_dma:
            # x / qkv weights reload; later readers get identical values
            nc.sync.dma_start(out=xr, in_=io["x_rot"])
            nc.sync.dma_start(out=wp[:, 0:5 * 128],
                              in_=io["wpack"][:, 0:5 * 128])
    for pe_ in pend:
        flush_o(*pe_)

    # ---- phase E: fused depthwise-conv + projection ----------------------
    shifts = [(dy, dx) for dy in (-1, 0, 1) for dx in (-1, 0, 1)]
    for cpair in range(2):
        psf = {}
        for cch in (2 * cpair, 2 * cpair + 1):
            psf[cch] = psum.tile([128, 512], F32, tag="s", bufs=3,
                                 name=f"psf{cch}")
        for widx in range(10):
            for cch in (2 * cpair, 2 * cpair + 1):
                if widx == 0:
                    lhsT = wblk(WPROJT)
                    rhs = attn[:, cch * 512:(cch + 1) * 512]
                else:
                    dy, dx = shifts[widx - 1]
                    lhsT = wblk(MS0 + widx - 1)
                    r0 = 1 + dy + 8 * cch
                    rhs = vpad3[:, r0:r0 + 8, 1 + dx:65 + dx]
                nc.tensor.matmul(psf[cch], lhsT=lhsT, rhs=rhs,
                                 start=(widx == 0), stop=(widx == 9))
        for idx, cch in enumerate((2 * cpair, 2 * cpair + 1)):
            sl = slice(cch * 512, (cch + 1) * 512)
            if idx:
                nc.scalar.copy(out=outsb[:, sl], in_=psf[cch])
            else:
                nc.vector.tensor_copy(out=outsb[:, sl], in_=psf[cch])
            if not no_dma:
                nc.sync.dma_start(out=io["out"][:, sl], in_=outsb[:, sl])

    # ---- reload proj/shift weights + consts for the next iteration -------
    if not no_dma:
        nc.sync.dma_start(out=wp[:, 5 * 128:NWBLK * 128],
                          in_=io["wpack"][:, 5 * 128:NWBLK * 128])
        nc.sync.dma_start(out=cst, in_=io["cst"])


def build_nc(reps=1):
    nc = bacc.Bacc(trn_type="TRN2", target_bir_lowering=False)
    io = {
        "wpack": nc.dram_tensor("wpack", [128, NWBLK * 128], BF16,
                                kind="ExternalInput").ap(),
        "cst": nc.dram_tensor("cst", [128, 4], F32,
                              kind="ExternalInput").ap(),
        "x_rot": nc.dram_tensor("x_rot", [128, N], BF16,
                                kind="ExternalInput").ap(),
        "out": nc.dram_tensor("out", [128, NHALF], F32,
                              kind="ExternalOutput").ap(),
    }
    with tile.TileContext(nc) as tc:
        pools, t = _alloc(tc)
        _emit_preamble(tc, t, io, pools)
        if reps == 1:
            _emit_body(tc, t, io, pools)
        else:
            with tc.For_i(0, reps, 1):
                _emit_body(tc, t, io, pools)
        for p in reversed(list(pools.values())):
            p.release()
    nc.compile()
    return nc


def host_prep(x, w_qkv, w_pe, w_proj):
    """Build the 8 per-core input maps from the full problem inputs."""
    bf16 = ml_dtypes.bfloat16
    x = np.ascontiguousarray(x, dtype=np.float32)
    wq = np.asarray(w_qkv, dtype=np.float32)[:, :, 0, 0]      # [256,128]
    wpe = np.asarray(w_pe, dtype=np.float32)[:, 0]            # [128,3,3]
    wpj = np.asarray(w_proj, dtype=np.float32)[:, :, 0, 0]    # [128,128]

    blocks = []
    for h in range(2):
        blocks.append(np.tile(wq[h * 128:h * 128 + 32], (4, 1)).T)       # WQh
    for h in range(2):
        blocks.append(np.tile(wq[h * 128 + 32:h * 128 + 64], (4, 1)).T)  # WKh
    blocks.insert(4, np.concatenate(
        [wq[64:128], wq[192:256]], axis=0).T)                 # WV01
    blocks.append(wpj.T)                                      # WPROJT
    for dy in (-1, 0, 1):
        for dx in (-1, 0, 1):
            blocks.append((wpj * wpe[:, dy + 1, dx + 1][None, :]).T)
    wpack = np.concatenate(blocks, axis=1).astype(bf16)       # [128, 15*128]

    in_maps = []
    for core in range(8):
        b, half = core // 2, core % 2
        y0 = 32 * half
        cst = np.zeros((128, 4), np.float32)
        cst[:, 0] = 1.0 if half == 1 else 0.0     # top halo valid?
        cst[:, 1] = 1.0 if half == 0 else 0.0     # bottom halo valid?
        cst[:, 2] = 1.0
        x_rot = np.roll(x[b], 1 - y0, axis=1).reshape(128, N).astype(bf16)
        in_maps.append({
            "wpack": np.ascontiguousarray(wpack),
            "cst": cst,
            "x_rot": np.ascontiguousarray(x_rot),
        })
    return in_maps


def assemble(results):
    out = np.zeros((B, C, H, W), np.float32)
    for core in range(8):
        b, half = core // 2, core % 2
        out[b, :, 32 * half:32 * half + 32, :] = \
            results[core]["out"].reshape(C, 32, W)
    return out


_NC_CACHE = {}


def _get_nc(reps=1):
    if reps not in _NC_CACHE:
        _NC_CACHE[reps] = build_nc(reps)
    return _NC_CACHE[reps]


def run(x, w_qkv, w_pe, w_proj, reps=1, **spmd_kwargs):
    nc = _get_nc(reps)
    in_maps = host_prep(x, w_qkv, w_pe, w_proj)
    res = run_bass_kernel_spmd(nc, in_maps, core_ids=list(range(8)),
                               **spmd_kwargs)
    return assemble(res.results), res


def kernel(x, w_qkv, w_pe, w_proj):
    out, _ = run(x, w_qkv, w_pe, w_proj)
    return out
